# revision 35
# baseline (speedup 1.0000x reference)
"""DFlashAttention Trainium2 kernel (8-core tensor-parallel over attention heads).

Shapes (hardcoded): D=2048, N=16 q-heads, K=8 kv-heads, H=128,
T_NOISE=2048 (query tokens), T_CTX=4096, S=6144 (kv tokens).

Sharding: core c owns q-heads {2c, 2c+1} and kv-head c (GQA groups=2).
Each core computes a partial (T, D) output (its 2 heads' slice of the
o-projection contraction); the host sums the 8 partials (TP unshard).

v2 layout strategy per core (all matmul operands bf16, fp32 PSUM):
  - x^T packed host-side as [128, 16 d-tiles, S]; ONE 2MB DMA per 512-token
    chunk (split across all 16 SDMA engines).
  - merged projection: per chunk, stationary = x-tile [128d, 128tok],
    moving = [Wk|Wv] (ctx chunks, 256) or [Wk|Wv|Wq01] (noise chunks, 512).
    Q projection rides the same x tiles => no separate phase B.
  - RMSNorm over H via ACT Square+accum_out; RoPE via on-device sin/cos
    (angle mod 2pi + range wrap + ACT Sin); tables built once for all 48
    token tiles; build overlaps phase A (pools never reuse its space).
  - per 128-token tile: norm+rope (fp32) -> bf16 -> PE transpose -> kT/qT.
  - attention in [s, t] orientation: scores^T = kT.T @ qT; exp on ACT
    (scale=1/sqrt(H) folded; no max subtraction, |score| <= 13.8, fp32-safe);
    probs bf16; row-sums via ones-matmul accumulated in PSUM; A@V accumulates
    over s-tiles in PSUM with V in natural [s, h] layout.
  - softmax division deferred past the o-projection (denominator constant
    along the contraction), applied as per-partition scalar multiply.
"""

import sys

for _p in ("/opt/trn_rl_repo", "/root/.axon_site/_ro/trn_rl_repo"):
    if _p not in sys.path:
        sys.path.append(_p)

import math
import numpy as np

import concourse.bass as bass
import concourse.tile as tile
from concourse import bacc
from concourse import mybir
from concourse.bass_utils import run_bass_kernel_spmd

D = 2048
N_HEADS = 16
K_HEADS = 8
H = 128
T_NOISE = 2048
T_CTX = 4096
S_ALL = T_CTX + T_NOISE          # 6144
EPS = 1e-6
ROPE_THETA = 1e6
N_CORES = 8
HEADS_PER_CORE = N_HEADS // N_CORES   # 2

P = 128                       # partition dim
S_TILES = S_ALL // P          # 48
T_TILES = T_NOISE // P        # 16
NOISE_TILE0 = T_CTX // P      # 32  (noise tokens are s-tiles 32..47)
D_TILES = D // P              # 16
FREE = 512                    # moving free-dim chunk
T_CHUNKS = T_NOISE // FREE    # 4
S_CHUNKS = S_ALL // FREE      # 12
NOISE_CHUNK0 = T_CTX // FREE  # 8
D_CHUNKS = D // FREE          # 4

F32 = mybir.dt.float32
BF16 = mybir.dt.bfloat16
MM_DT = BF16                  # dtype for all matmul operands

TWO_PI = 2.0 * math.pi
INV_SQRT_H = 1.0 / math.sqrt(H)
HALF = H // 2

# consts tensor layout (fp32, [P, CONST_W])
CO_INVF = 0                  # [P, 64]   inv_freq broadcast
CO_QSC = 64                  # [P, 128]  q_scale broadcast
CO_KSC = 192                 # [P, 128]  k_scale broadcast
CO_POS = 320                 # [P, 48]   positions, tile-major columns
CO_ONES = 368                # [P, 128]  fp32 ones (bcast-matmul stationary)
CONST_W = 496

_CACHE = {}


def _build_program(reps=1):
    nc = bacc.Bacc("TRN2", target_bir_lowering=False, debug=False,
                   num_devices=N_CORES)

    xTr = nc.dram_tensor("xTr", [P, D_TILES, S_ALL], MM_DT,
                         kind="ExternalInput").ap()
    wkvq = nc.dram_tensor("wkvq", [P, D_TILES * 4 * H], MM_DT,
                          kind="ExternalInput").ap()
    wo = nc.dram_tensor("wo", [P, HEADS_PER_CORE * D], MM_DT,
                        kind="ExternalInput").ap()
    constsf = nc.dram_tensor("constsf", [P, CONST_W], F32,
                             kind="ExternalInput").ap()
    constsb = nc.dram_tensor("constsb", [P, 2 * P], MM_DT,
                             kind="ExternalInput").ap()
    out = nc.dram_tensor("out", [T_NOISE, D], MM_DT, kind="ExternalOutput").ap()

    with tile.TileContext(nc) as tc:
        for rep in range(reps):
            _emit(nc, tc, xTr, wkvq, wo, constsf, constsb, out,
                  pfx=f"r{rep}_")
    nc.compile()
    return nc, "out"


def _emit(nc, tc, xTr, wkvq, wo, constsf, constsb, out, pfx=""):
    import contextlib
    ctx = contextlib.ExitStack()
    with ctx:
        const = ctx.enter_context(tc.tile_pool(name=pfx + "const", bufs=1))
        persist = ctx.enter_context(tc.tile_pool(name=pfx + "persist", bufs=1))

        # ---- constants (3 DMAs total) ----
        cf = const.tile([P, CONST_W], F32, tag="cf")
        nc.sync.dma_start(cf[:], constsf[:])
        cb = const.tile([P, 2 * P], MM_DT, tag="cb")
        nc.sync.dma_start(cb[:], constsb[:])
        ident = cb[:, 0:P]
        # full [128,128] all-ones stationary for row-sums: M=128 output (all
        # rows equal) avoids the ~100ns PE reconfig cost of M=1 matmuls
        ones = cb[:, P:2 * P]
        invf_sb = cf[:, CO_INVF:CO_INVF + HALF]
        qsc_sb = cf[:, CO_QSC:CO_QSC + H]
        ksc_sb = cf[:, CO_KSC:CO_KSC + H]
        pos_sb = cf[:, CO_POS:CO_POS + S_TILES]
        eps_col = const.tile([P, 1], F32, tag="eps")
        nc.vector.memset(eps_col[:], EPS)

        wkvq_sb = const.tile([P, D_TILES * 4 * H], MM_DT, tag="wkvq")
        nc.sync.dma_start(wkvq_sb[:], wkvq[:])
        # wo is only needed in the o-projection; its DMA is emitted after
        # phase A so it doesn't delay the first x-chunk on the sync queue.
        wo_sb = const.tile([P, HEADS_PER_CORE * D], MM_DT, tag="wo")

        # ---- persistent activations ----
        sin_all = persist.tile([P, S_TILES * HALF], F32, tag="sin")
        cos_all = persist.tile([P, S_TILES * HALF], F32, tag="cos")
        kT_sb = persist.tile([P, S_ALL], MM_DT, tag="kT")
        v_sb = persist.tile([P, S_ALL], MM_DT, tag="v")       # [s-tile, h] blocks
        qT_sb = persist.tile([P, HEADS_PER_CORE * T_NOISE], MM_DT, tag="qT")
        oT_sb = persist.tile([P, HEADS_PER_CORE * T_NOISE], MM_DT, tag="oT")
        r_all = persist.tile([1, HEADS_PER_CORE * T_NOISE], F32, tag="r")
        rcol = persist.tile([P, HEADS_PER_CORE * T_TILES], F32, tag="rcol")
        rrow_inv = persist.tile([1, HEADS_PER_CORE * FREE], F32, tag="rri")
        oTn2 = persist.tile([P, HEADS_PER_CORE * FREE], MM_DT, tag="oTn")

        # ---- RoPE sin/cos tables for all 48 token tiles ----
        # angle = pos * inv_freq; range-reduce mod 2pi via Cody-Waite
        # (k = int(angle/2pi); red = ((ang - k*c1) - k*c2) - k*c3).
        # Pool stays open for the whole kernel so phase A never waits on a
        # space-reuse (WAR) dependency against the build. Tables are stored
        # in BUILD order (noise tiles first, matching the noise-first chunk
        # order); `simap` maps token-tile index -> table column block.
        CW1, CW2, CW3 = 6.28125, 0.0019353071693331003, 1.0253131677018246e-11
        BUILD_G0 = list(range(NOISE_TILE0, S_TILES)) + list(range(0, 8))
        BUILD_G1 = list(range(8, NOISE_TILE0))
        simap = {}
        for idx, si in enumerate(BUILD_G0 + BUILD_G1):
            simap[si] = idx
        rp = ctx.enter_context(tc.tile_pool(name=pfx + "ropebuild", bufs=1))

        def emit_rope_build(tiles, dst0):
            ng = len(tiles)
            ang = rp.tile([P, ng * HALF], F32, tag="ang", name="ang")
            kq = rp.tile([P, ng * HALF], F32, tag="kq", name="kq")
            ki = rp.tile([P, ng * HALF], mybir.dt.int32, tag="ki", name="ki")
            wrap = rp.tile([P, ng * HALF], F32, tag="wrap", name="wrap")
            for j, si in enumerate(tiles):
                nc.vector.tensor_scalar_mul(
                    ang[:, j * HALF:(j + 1) * HALF], invf_sb,
                    pos_sb[:, si:si + 1])
            nc.vector.tensor_scalar_mul(kq[:], ang[:], 1.0 / TWO_PI)
            nc.vector.tensor_copy(ki[:], kq[:])
            nc.vector.tensor_copy(kq[:], ki[:])
            nc.vector.cody_waite_cascade(ang[:], ang[:], kq[:], CW1, CW2, CW3)
            dst = slice(dst0 * HALF, (dst0 + ng) * HALF)
            nc.vector.add_range_wrap(wrap[:], ang[:], 0.0, math.pi, TWO_PI)
            nc.scalar.activation(sin_all[:, dst], wrap[:],
                                 mybir.ActivationFunctionType.Sin)
            nc.vector.add_range_wrap(wrap[:], ang[:], math.pi / 2, math.pi,
                                     TWO_PI)
            nc.scalar.activation(cos_all[:, dst], wrap[:],
                                 mybir.ActivationFunctionType.Sin)

        emit_rope_build(BUILD_G0, 0)

        def norm_rope(src_psum, scale_sb, si, work, tag):
            """src_psum [P(tok),H] fp32 -> rms-norm*scale -> rope -> bf16
            xr tile. si = token-tile index for positions."""
            sq = work.tile([P, H], F32, tag="sq")
            ssq = work.tile([P, 1], F32, tag="ssq")
            nc.scalar.activation(sq[:], src_psum,
                                 mybir.ActivationFunctionType.Square,
                                 accum_out=ssq[:])
            rms = work.tile([P, 1], F32, tag="rms")
            nc.scalar.activation(rms[:], ssq[:],
                                 mybir.ActivationFunctionType.Sqrt,
                                 bias=eps_col[:], scale=1.0 / H)
            rinv = work.tile([P, 1], F32, tag="rinv")
            nc.vector.reciprocal(rinv[:], rms[:])
            xn = work.tile([P, H], F32, tag="xn")
            nc.vector.scalar_tensor_tensor(
                xn[:], src_psum, rinv[:], scale_sb,
                mybir.AluOpType.mult, mybir.AluOpType.mult)
            # rope
            bi = simap[si]
            co = cos_all[:, bi * HALF:(bi + 1) * HALF]
            sn = sin_all[:, bi * HALF:(bi + 1) * HALF]
            x1 = xn[:, 0:HALF]
            x2 = xn[:, HALF:H]
            t1 = work.tile([P, HALF], F32, tag="t1")
            t2 = work.tile([P, HALF], F32, tag="t2")
            xr = work.tile([P, H], MM_DT, tag="xr" + tag)
            nc.vector.tensor_mul(t1[:], x1, co)
            nc.vector.tensor_mul(t2[:], x2, sn)
            nc.vector.tensor_sub(xr[:, 0:HALF], t1[:], t2[:])
            nc.vector.tensor_mul(t1[:], x2, co)
            nc.vector.tensor_mul(t2[:], x1, sn)
            nc.vector.tensor_add(xr[:, HALF:H], t1[:], t2[:])
            return xr

        def transpose_out(pairs, psum_t):
            """Batch PE transposes back-to-back (each is_transpose mode
            switch costs ~120ns); evacuate on ACT (DVE is A's bottleneck)."""
            pts = []
            for i, (xr, dst_sb) in enumerate(pairs):
                pt = psum_t.tile([P, P], MM_DT, tag=f"pt{i}")
                nc.tensor.transpose(pt[:], xr[:], ident)
                pts.append(pt)
            for (xr, dst_sb), pt in zip(pairs, pts):
                nc.scalar.copy(dst_sb, pt[:])

        # ---- Phase A: merged K/V/Q projection, norm+rope, transposes ----
        # Noise chunks (sc >= 8) also produce the 2 q-heads from the same
        # staged x tiles: moving = [Wk|Wv|Wq0|Wq1] (512 wide).
        xp = ctx.enter_context(tc.tile_pool(name=pfx + "pa_x", bufs=3))
        work = ctx.enter_context(tc.tile_pool(name=pfx + "pa_w", bufs=2))
        # Noise chunks first: their triple (k,q0,q1) norm chains are the DVE
        # bottleneck, so front-load them; the lighter ctx chains then gate
        # the A->C transition. Rope-table group 1 (ctx tiles 8..31) is built
        # after 2 ctx chunks so early chains aren't queued behind it on DVE.
        CHUNK_ORDER = list(range(NOISE_CHUNK0, S_CHUNKS)) + \
            list(range(0, NOISE_CHUNK0))
        with tc.tile_pool(name=pfx + "pa_ps", bufs=2, space="PSUM") as pskv, \
             tc.tile_pool(name=pfx + "pa_pt", bufs=1, space="PSUM") as pst:
            # PE warm-up spin while the first x-chunk DMA is in flight:
            # ~3.5us of sustained matmuls flips the HAM clock-gate to 8/8
            # before the real projection matmuls arrive.
            warm = pskv.tile([P, 4 * H], F32, tag="kv0", name="pswarm")
            for _ in range(32):
                nc.tensor.matmul(warm[:, 0:P], ident, ident,
                                 start=True, stop=True)
            for ci, sc in enumerate(CHUNK_ORDER):
                if ci == 6:
                    emit_rope_build(BUILD_G1, len(BUILD_G0))
                noise = sc >= NOISE_CHUNK0
                W = 4 * H if noise else 2 * H
                xt = xp.tile([P, D_TILES, FREE], MM_DT, tag="xstage",
                             name="xstage")
                nc.sync.dma_start(
                    xt[:], xTr[:, :, sc * FREE:(sc + 1) * FREE])
                # two half-chunks of 256 tokens, each double-buffered in PSUM
                for half in range(2):
                    ps = [pskv.tile([P, 4 * H], F32, tag=f"kv{j}",
                                    name=f"pskv{j}") for j in range(2)]
                    for d in range(D_TILES):
                        wslice = wkvq_sb[:, d * 4 * H:d * 4 * H + W]
                        for j in range(2):
                            tok0 = half * 2 * P + j * P
                            nc.tensor.matmul(
                                ps[j][:, 0:W],
                                xt[:, d, tok0:tok0 + P],
                                wslice,
                                start=(d == 0), stop=(d == D_TILES - 1))
                    for j in range(2):
                        si = sc * 4 + half * 2 + j
                        nc.scalar.copy(
                            v_sb[:, si * P:(si + 1) * P], ps[j][:, H:2 * H])
                        pairs = [(norm_rope(ps[j][:, 0:H], ksc_sb, si,
                                            work, "k"),
                                  kT_sb[:, si * P:(si + 1) * P])]
                        if noise:
                            ti = si - NOISE_TILE0
                            for hh in range(HEADS_PER_CORE):
                                pairs.append((
                                    norm_rope(ps[j][:, (2 + hh) * H:(3 + hh) * H],
                                              qsc_sb, si, work, f"q{hh}"),
                                    qT_sb[:, hh * T_NOISE + ti * P:
                                          hh * T_NOISE + (ti + 1) * P]))
                        transpose_out(pairs, pst)

        nc.sync.dma_start(wo_sb[:], wo[:])

        # ---- Phase C+D fused: attention, then per-t-chunk o-projection ----
        # Software-pipelined with LAG=2: AV/rowsum of score-pair p-2 are
        # emitted AFTER the score matmuls of pair p, so the in-order PE queue
        # never stalls waiting for exp on ACT (keeps HAM at K=8/8 too).
        # After both heads of a t-chunk finish, that chunk's o-projection
        # (phase D work) is drip-fed into the PE stream one t-tile at a time.
        # PSUM budget: scores 2x2 + av 1 + rr 1 + po 2 = 8 banks exactly.
        PAIR = 2 * FREE   # exp processes two score banks at once
        NSP = S_TILES // 2
        LAG = 2
        pexp = ctx.enter_context(tc.tile_pool(name=pfx + "pc_exp", bufs=4))
        dwork = ctx.enter_context(tc.tile_pool(name=pfx + "pd_w", bufs=2))
        with tc.tile_pool(name=pfx + "pc_av", bufs=1, space="PSUM") as pav, \
             tc.tile_pool(name=pfx + "pc_r", bufs=1, space="PSUM") as pr, \
             tc.tile_pool(name=pfx + "pd_ps", bufs=1, space="PSUM") as pso:

            def emit_avrr(st):
                ex, sp, av, rr = st["ex"], st["sp"], st["av"], st["rr"]
                for u in range(2):
                    si = sp * 2 + u
                    nc.tensor.matmul(
                        av[:], v_sb[:, si * P:(si + 1) * P],
                        ex[:, u * FREE:(u + 1) * FREE],
                        start=(si == 0), stop=(si == S_TILES - 1))
                for u in range(2):
                    si = sp * 2 + u
                    nc.tensor.matmul(
                        rr[:], ones,
                        ex[:, u * FREE:(u + 1) * FREE],
                        start=(si == 0), stop=(si == S_TILES - 1))

            def finish_chunk(st):
                hh, tch = st["hh"], st["tch"]
                nc.vector.tensor_copy(
                    oT_sb[:, hh * T_NOISE + tch * FREE:
                          hh * T_NOISE + (tch + 1) * FREE], st["av"][:])
                nc.vector.tensor_copy(
                    r_all[0:1, hh * T_NOISE + tch * FREE:
                          hh * T_NOISE + (tch + 1) * FREE], st["rr"][0:1, :])

            def prep_rcol(tch):
                # raw row-sums -> per-partition columns (tiny SBUF->SBUF DMA
                # transposes), then one cheap reciprocal per head slice
                # scalar (ACT) hwdge queue: keeps these off the sync queue
                # where the output DMAs live
                for hh in range(HEADS_PER_CORE):
                    for ti in range(tch * 4, tch * 4 + 4):
                        nc.scalar.dma_start(
                            rcol[:, hh * T_TILES + ti:hh * T_TILES + ti + 1],
                            r_all[0:1, hh * T_NOISE + ti * P:
                                  hh * T_NOISE + (ti + 1) * P])
                    c0 = hh * T_TILES + tch * 4
                    nc.vector.reciprocal(rcol[:, c0:c0 + 4], rcol[:, c0:c0 + 4])

            def emit_oproj_unit(ti, dc, pool=None, tail=False):
                # o-projection for one (128-token tile, 512-col d-chunk):
                # tmp = po1 * r1; ot = po0 * r0 + tmp; bf16 out.
                # Units are dripped >= 2 score-pairs apart so the pso bufs=1
                # WAR (next unit's matmul vs this unit's DVE reads) clears.
                # In the tail, tmp moves to ACT (idle there) and po comes from
                # the 4-bank ptail pool so units pipeline.
                pool = pool if pool is not None else pso
                po = [pool.tile([P, FREE], F32, tag=f"po{h}", name=f"po{h}")
                      for h in range(HEADS_PER_CORE)]
                for h in range(HEADS_PER_CORE):
                    nc.tensor.matmul(
                        po[h][:],
                        oT_sb[:, h * T_NOISE + ti * P:h * T_NOISE + (ti + 1) * P],
                        wo_sb[:, h * D + dc * FREE:h * D + (dc + 1) * FREE],
                        start=True, stop=True)
                tmp = dwork.tile([P, FREE], F32, tag="tmp")
                if tail:
                    nc.scalar.activation(
                        tmp[:], po[1][:], mybir.ActivationFunctionType.Copy,
                        scale=rcol[:, T_TILES + ti:T_TILES + ti + 1])
                else:
                    nc.vector.tensor_scalar_mul(
                        tmp[:], po[1][:],
                        rcol[:, T_TILES + ti:T_TILES + ti + 1])
                ot = dwork.tile([P, FREE], MM_DT, tag="ot")
                nc.vector.scalar_tensor_tensor(
                    ot[:], po[0][:], rcol[:, ti:ti + 1], tmp[:],
                    mybir.AluOpType.mult, mybir.AluOpType.add)
                nc.sync.dma_start(
                    out[ti * P:(ti + 1) * P, dc * FREE:(dc + 1) * FREE],
                    ot[:])

            pending = []          # score-pairs awaiting AV/rowsum emission
            dqueue = []           # o-projection (ti, dc) units awaiting emission

            def retire_one():
                st = pending.pop(0)
                emit_avrr(st)
                if st["sp"] == NSP - 1:
                    finish_chunk(st)
                    if st["hh"] == HEADS_PER_CORE - 1:
                        prep_rcol(st["tch"])
                        dqueue.extend(
                            (ti, dc)
                            for ti in range(st["tch"] * 4, st["tch"] * 4 + 4)
                            for dc in range(D_CHUNKS))

            gpair = 0
            with tc.tile_pool(name=pfx + "pc_sc", bufs=2,
                              space="PSUM") as psc:
                for tch in range(T_CHUNKS):
                    for hh in range(HEADS_PER_CORE):
                        qslice = qT_sb[:, hh * T_NOISE + tch * FREE:
                                       hh * T_NOISE + (tch + 1) * FREE]
                        av = pav.tile([P, FREE], F32, tag="av")
                        rr = pr.tile([P, FREE], F32, tag="rr")
                        for sp in range(NSP):
                            sc_ps = psc.tile([P, PAIR], F32, tag="sc")
                            ex = pexp.tile([P, PAIR], MM_DT, tag="ex")
                            for u in range(2):
                                si = sp * 2 + u
                                nc.tensor.matmul(
                                    sc_ps[:, u * FREE:(u + 1) * FREE],
                                    kT_sb[:, si * P:(si + 1) * P], qslice,
                                    start=True, stop=True)
                            if len(pending) >= LAG:
                                retire_one()
                            if dqueue and gpair % 2 == 0:
                                emit_oproj_unit(*dqueue.pop(0))
                            nc.scalar.activation(
                                ex[:], sc_ps[:],
                                mybir.ActivationFunctionType.Exp,
                                scale=INV_SQRT_H)
                            pending.append(dict(ex=ex, sp=sp, av=av, rr=rr,
                                                hh=hh, tch=tch))
                            gpair += 1
                while pending:
                    retire_one()
            # ---- fast tail (last t-chunk's o-projection) ----
            # Pre-normalize oT with a PE row-broadcast of 1/rowsum so each
            # unit is just 2 accumulating matmuls into ONE bank + an ACT
            # copy — no per-unit DVE chain. Units rotate over 4 banks
            # (freed score banks) so they fully pipeline.
            with tc.tile_pool(name=pfx + "pd_tail", bufs=1,
                              space="PSUM") as ptail:
                ltch = T_CHUNKS - 1
                leaked = [u for u in dqueue if u[0] < ltch * 4]
                for ti, dc in leaked:
                    emit_oproj_unit(ti, dc, pool=ptail, tail=True)
                onesrow = cf[0:1, CO_ONES:CO_ONES + P]
                for hh in range(HEADS_PER_CORE):
                    # rcol inverse columns -> row layout (tiny DMA transposes)
                    for k in range(4):
                        ti = ltch * 4 + k
                        nc.scalar.dma_start(
                            rrow_inv[0:1, hh * FREE + k * P:
                                     hh * FREE + (k + 1) * P],
                            rcol[:, hh * T_TILES + ti:hh * T_TILES + ti + 1])
                    rbc_ps = ptail.tile([P, FREE], F32, tag=f"rbc{hh}")
                    nc.tensor.matmul(
                        rbc_ps[:], onesrow,
                        rrow_inv[0:1, hh * FREE:(hh + 1) * FREE],
                        start=True, stop=True)
                    rbc_sb = persist.tile([P, FREE], F32, tag="rbc")
                    nc.scalar.copy(rbc_sb[:], rbc_ps[:])
                    nc.vector.tensor_mul(
                        oTn2[:, hh * FREE:(hh + 1) * FREE],
                        oT_sb[:, hh * T_NOISE + ltch * FREE:
                              hh * T_NOISE + (ltch + 1) * FREE],
                        rbc_sb[:])
                for idx, (ti, dc) in enumerate(u for u in dqueue
                                               if u[0] >= ltch * 4):
                    k = ti - ltch * 4
                    po = ptail.tile([P, FREE], F32, tag=f"tp{idx % 2}",
                                    name="tpo")
                    for h in range(HEADS_PER_CORE):
                        nc.tensor.matmul(
                            po[:],
                            oTn2[:, h * FREE + k * P:h * FREE + (k + 1) * P],
                            wo_sb[:, h * D + dc * FREE:h * D + (dc + 1) * FREE],
                            start=(h == 0), stop=(h == HEADS_PER_CORE - 1))
                    ot = dwork.tile([P, FREE], MM_DT, tag="ot")
                    nc.scalar.copy(ot[:], po[:])
                    nc.sync.dma_start(
                        out[ti * P:(ti + 1) * P, dc * FREE:(dc + 1) * FREE],
                        ot[:])


def _get_program(reps=1):
    key = f"prog{reps}"
    if key not in _CACHE:
        _CACHE[key] = _build_program(reps)
    return _CACHE[key]


def prepare_in_maps(x_noise, target_hidden, Wq, Wk, Wv, Wo, q_scale, k_scale,
                    noise_positions, ctx_positions):
    import ml_dtypes
    bf16 = ml_dtypes.bfloat16

    x_noise = np.asarray(x_noise, dtype=np.float32)
    target_hidden = np.asarray(target_hidden, dtype=np.float32)
    Wq = np.asarray(Wq, dtype=np.float32)
    Wk = np.asarray(Wk, dtype=np.float32)
    Wv = np.asarray(Wv, dtype=np.float32)
    Wo = np.asarray(Wo, dtype=np.float32)
    q_scale = np.asarray(q_scale, dtype=np.float32)
    k_scale = np.asarray(k_scale, dtype=np.float32)

    x_all = np.concatenate([target_hidden, x_noise], axis=0)       # (S, D)
    # xTr[p, dt, s] = x_all[s, dt*128+p]
    xTr = np.ascontiguousarray(
        x_all.T.reshape(D_TILES, P, S_ALL).transpose(1, 0, 2)).astype(bf16)

    pos_all = np.concatenate(
        [np.asarray(ctx_positions), np.asarray(noise_positions)]
    ).astype(np.float32)
    # pos columns: consts[:, CO_POS + si] = pos of tile si (same for all p? no:
    # partition p holds position of token si*128+p)
    posT = np.ascontiguousarray(pos_all.reshape(S_TILES, P).T)     # (P, 48)
    inv_freq = (ROPE_THETA ** (-np.arange(HALF, dtype=np.float32) * 2.0 / H)
                ).astype(np.float32)
    constsf = np.zeros((P, CONST_W), dtype=np.float32)
    constsf[:, CO_INVF:CO_INVF + HALF] = inv_freq
    constsf[:, CO_QSC:CO_QSC + H] = q_scale
    constsf[:, CO_KSC:CO_KSC + H] = k_scale
    constsf[:, CO_POS:CO_POS + S_TILES] = posT
    constsf[:, CO_ONES:CO_ONES + P] = 1.0

    constsb = np.zeros((P, 2 * P), dtype=np.float32)
    constsb[:, 0:P] = np.eye(P, dtype=np.float32)
    constsb[:, P:2 * P] = 1.0
    constsb = constsb.astype(bf16)

    in_maps = []
    for c in range(N_CORES):
        # wkvq[p, dt*512 + col] = W*[dt*128+p, col] with cols [k|v|q0|q1]
        wkvq = np.concatenate([
            Wk[:, c, :], Wv[:, c, :],
            Wq[:, 2 * c, :], Wq[:, 2 * c + 1, :]], axis=1)          # (D, 512)
        wkvq = np.ascontiguousarray(
            wkvq.reshape(D_TILES, P, 4 * H).transpose(1, 0, 2)
            .reshape(P, D_TILES * 4 * H)).astype(bf16)
        # wo[p, h*D + dcol] = Wo[2c+h, p, dcol]
        woc = np.ascontiguousarray(
            Wo[2 * c:2 * c + 2].transpose(1, 0, 2).reshape(P, 2 * D)
        ).astype(bf16)
        in_maps.append({
            "xTr": xTr, "wkvq": wkvq, "wo": woc,
            "constsf": constsf, "constsb": constsb,
        })
    return in_maps


def kernel(**inputs):
    in_maps = prepare_in_maps(**inputs)
    nc, out_name = _get_program()
    res = run_bass_kernel_spmd(nc, in_maps, core_ids=list(range(N_CORES)))
    acc = np.zeros((T_NOISE, D), dtype=np.float32)
    for r in res.results:
        acc += np.asarray(r[out_name], dtype=np.float32)
    return acc


def run_traced(inputs, **kw):
    """Run once with NTFF tracing; returns BassKernelResults (exec_time_ns)."""
    in_maps = prepare_in_maps(**inputs)
    nc, out_name = _get_program()
    return run_bass_kernel_spmd(nc, in_maps, core_ids=list(range(N_CORES)),
                                trace=True, **kw)


# revision 37
# speedup vs baseline: 1.0285x; 1.0285x over previous
"""DFlashAttention Trainium2 kernel (8-core tensor-parallel over attention heads).

Shapes (hardcoded): D=2048, N=16 q-heads, K=8 kv-heads, H=128,
T_NOISE=2048 (query tokens), T_CTX=4096, S=6144 (kv tokens).

Sharding: core c owns q-heads {2c, 2c+1} and kv-head c (GQA groups=2).
Each core computes a partial (T, D) output (its 2 heads' slice of the
o-projection contraction); the host sums the 8 partials (TP unshard).

v2 layout strategy per core (all matmul operands bf16, fp32 PSUM):
  - x^T packed host-side as [128, 16 d-tiles, S]; ONE 2MB DMA per 512-token
    chunk (split across all 16 SDMA engines).
  - merged projection: per chunk, stationary = x-tile [128d, 128tok],
    moving = [Wk|Wv] (ctx chunks, 256) or [Wk|Wv|Wq01] (noise chunks, 512).
    Q projection rides the same x tiles => no separate phase B.
  - RMSNorm over H via ACT Square+accum_out; RoPE via on-device sin/cos
    (angle mod 2pi + range wrap + ACT Sin); tables built once for all 48
    token tiles; build overlaps phase A (pools never reuse its space).
  - per 128-token tile: norm+rope (fp32) -> bf16 -> PE transpose -> kT/qT.
  - attention in [s, t] orientation: scores^T = kT.T @ qT; exp on ACT
    (scale=1/sqrt(H) folded; no max subtraction, |score| <= 13.8, fp32-safe);
    probs bf16; row-sums via ones-matmul accumulated in PSUM; A@V accumulates
    over s-tiles in PSUM with V in natural [s, h] layout.
  - softmax division deferred past the o-projection (denominator constant
    along the contraction), applied as per-partition scalar multiply.
"""

import sys

for _p in ("/opt/trn_rl_repo", "/root/.axon_site/_ro/trn_rl_repo"):
    if _p not in sys.path:
        sys.path.append(_p)

import math
import numpy as np

import concourse.bass as bass
import concourse.tile as tile
from concourse import bacc
from concourse import mybir
from concourse.bass_utils import run_bass_kernel_spmd

D = 2048
N_HEADS = 16
K_HEADS = 8
H = 128
T_NOISE = 2048
T_CTX = 4096
S_ALL = T_CTX + T_NOISE          # 6144
EPS = 1e-6
ROPE_THETA = 1e6
N_CORES = 8
HEADS_PER_CORE = N_HEADS // N_CORES   # 2

P = 128                       # partition dim
S_TILES = S_ALL // P          # 48
T_TILES = T_NOISE // P        # 16
NOISE_TILE0 = T_CTX // P      # 32  (noise tokens are s-tiles 32..47)
D_TILES = D // P              # 16
FREE = 512                    # moving free-dim chunk
T_CHUNKS = T_NOISE // FREE    # 4
S_CHUNKS = S_ALL // FREE      # 12
NOISE_CHUNK0 = T_CTX // FREE  # 8
D_CHUNKS = D // FREE          # 4

F32 = mybir.dt.float32
BF16 = mybir.dt.bfloat16
MM_DT = BF16                  # dtype for all matmul operands

TWO_PI = 2.0 * math.pi
INV_SQRT_H = 1.0 / math.sqrt(H)
HALF = H // 2

# consts tensor layout (fp32, [P, CONST_W])
CO_INVF = 0                  # [P, 64]   inv_freq broadcast
CO_QSC = 64                  # [P, 128]  q_scale broadcast
CO_KSC = 192                 # [P, 128]  k_scale broadcast
CO_POS = 320                 # [P, 48]   positions, tile-major columns
CONST_W = 368

_CACHE = {}


def _build_program(reps=1):
    nc = bacc.Bacc("TRN2", target_bir_lowering=False, debug=False,
                   num_devices=N_CORES)

    xTr = nc.dram_tensor("xTr", [P, D_TILES, S_ALL], MM_DT,
                         kind="ExternalInput").ap()
    wkvq = nc.dram_tensor("wkvq", [P, D_TILES * 4 * H], MM_DT,
                          kind="ExternalInput").ap()
    wo = nc.dram_tensor("wo", [P, HEADS_PER_CORE * D], MM_DT,
                        kind="ExternalInput").ap()
    constsf = nc.dram_tensor("constsf", [P, CONST_W], F32,
                             kind="ExternalInput").ap()
    constsb = nc.dram_tensor("constsb", [P, 2 * P], MM_DT,
                             kind="ExternalInput").ap()
    out = nc.dram_tensor("out", [T_NOISE, D], MM_DT, kind="ExternalOutput").ap()

    with tile.TileContext(nc) as tc:
        for rep in range(reps):
            _emit(nc, tc, xTr, wkvq, wo, constsf, constsb, out,
                  pfx=f"r{rep}_")
    nc.compile()
    return nc, "out"


def _emit(nc, tc, xTr, wkvq, wo, constsf, constsb, out, pfx=""):
    import contextlib
    ctx = contextlib.ExitStack()
    with ctx:
        const = ctx.enter_context(tc.tile_pool(name=pfx + "const", bufs=1))
        persist = ctx.enter_context(tc.tile_pool(name=pfx + "persist", bufs=1))

        # ---- constants (3 DMAs total) ----
        cf = const.tile([P, CONST_W], F32, tag="cf")
        nc.sync.dma_start(cf[:], constsf[:])
        cb = const.tile([P, 2 * P], MM_DT, tag="cb")
        nc.sync.dma_start(cb[:], constsb[:])
        ident = cb[:, 0:P]
        # full [128,128] all-ones stationary for row-sums: M=128 output (all
        # rows equal) avoids the ~100ns PE reconfig cost of M=1 matmuls
        ones = cb[:, P:2 * P]
        invf_sb = cf[:, CO_INVF:CO_INVF + HALF]
        qsc_sb = cf[:, CO_QSC:CO_QSC + H]
        ksc_sb = cf[:, CO_KSC:CO_KSC + H]
        pos_sb = cf[:, CO_POS:CO_POS + S_TILES]
        eps_col = const.tile([P, 1], F32, tag="eps")
        nc.vector.memset(eps_col[:], EPS)

        wkvq_sb = const.tile([P, D_TILES * 4 * H], MM_DT, tag="wkvq")
        nc.sync.dma_start(wkvq_sb[:], wkvq[:])
        # wo is only needed in the o-projection; its DMA is emitted after
        # phase A so it doesn't delay the first x-chunk on the sync queue.
        wo_sb = const.tile([P, HEADS_PER_CORE * D], MM_DT, tag="wo")

        # ---- persistent activations ----
        sin_all = persist.tile([P, S_TILES * HALF], F32, tag="sin")
        cos_all = persist.tile([P, S_TILES * HALF], F32, tag="cos")
        kT_sb = persist.tile([P, S_ALL], MM_DT, tag="kT")
        v_sb = persist.tile([P, S_ALL], MM_DT, tag="v")       # [s-tile, h] blocks
        qT_sb = persist.tile([P, HEADS_PER_CORE * T_NOISE], MM_DT, tag="qT")
        oT_sb = persist.tile([P, HEADS_PER_CORE * T_NOISE], MM_DT, tag="oT")
        r_all = persist.tile([1, HEADS_PER_CORE * T_NOISE], F32, tag="r")
        rcol = persist.tile([P, HEADS_PER_CORE * T_TILES], F32, tag="rcol")

        # ---- RoPE sin/cos tables for all 48 token tiles ----
        # angle = pos * inv_freq; range-reduce mod 2pi via Cody-Waite
        # (k = int(angle/2pi); red = ((ang - k*c1) - k*c2) - k*c3).
        # Pool stays open for the whole kernel so phase A never waits on a
        # space-reuse (WAR) dependency against the build. Tables are stored
        # in BUILD order (noise tiles first, matching the noise-first chunk
        # order); `simap` maps token-tile index -> table column block.
        CW1, CW2, CW3 = 6.28125, 0.0019353071693331003, 1.0253131677018246e-11
        BUILD_G0 = list(range(NOISE_TILE0, S_TILES)) + list(range(0, 8))
        BUILD_G1 = list(range(8, NOISE_TILE0))
        simap = {}
        for idx, si in enumerate(BUILD_G0 + BUILD_G1):
            simap[si] = idx
        rp = ctx.enter_context(tc.tile_pool(name=pfx + "ropebuild", bufs=1))

        def emit_rope_build(tiles, dst0):
            ng = len(tiles)
            ang = rp.tile([P, ng * HALF], F32, tag="ang", name="ang")
            kq = rp.tile([P, ng * HALF], F32, tag="kq", name="kq")
            ki = rp.tile([P, ng * HALF], mybir.dt.int32, tag="ki", name="ki")
            wrap = rp.tile([P, ng * HALF], F32, tag="wrap", name="wrap")
            for j, si in enumerate(tiles):
                nc.vector.tensor_scalar_mul(
                    ang[:, j * HALF:(j + 1) * HALF], invf_sb,
                    pos_sb[:, si:si + 1])
            nc.vector.tensor_scalar_mul(kq[:], ang[:], 1.0 / TWO_PI)
            nc.vector.tensor_copy(ki[:], kq[:])
            nc.vector.tensor_copy(kq[:], ki[:])
            nc.vector.cody_waite_cascade(ang[:], ang[:], kq[:], CW1, CW2, CW3)
            dst = slice(dst0 * HALF, (dst0 + ng) * HALF)
            nc.vector.add_range_wrap(wrap[:], ang[:], 0.0, math.pi, TWO_PI)
            nc.scalar.activation(sin_all[:, dst], wrap[:],
                                 mybir.ActivationFunctionType.Sin)
            nc.vector.add_range_wrap(wrap[:], ang[:], math.pi / 2, math.pi,
                                     TWO_PI)
            nc.scalar.activation(cos_all[:, dst], wrap[:],
                                 mybir.ActivationFunctionType.Sin)

        emit_rope_build(BUILD_G0, 0)

        def norm_rope(src_psum, scale_sb, si, work, tag):
            """src_psum [P(tok),H] fp32 -> rms-norm*scale -> rope -> bf16
            xr tile. si = token-tile index for positions."""
            sq = work.tile([P, H], F32, tag="sq")
            ssq = work.tile([P, 1], F32, tag="ssq")
            nc.scalar.activation(sq[:], src_psum,
                                 mybir.ActivationFunctionType.Square,
                                 accum_out=ssq[:])
            rms = work.tile([P, 1], F32, tag="rms")
            nc.scalar.activation(rms[:], ssq[:],
                                 mybir.ActivationFunctionType.Sqrt,
                                 bias=eps_col[:], scale=1.0 / H)
            rinv = work.tile([P, 1], F32, tag="rinv")
            nc.vector.reciprocal(rinv[:], rms[:])
            xn = work.tile([P, H], F32, tag="xn")
            nc.vector.scalar_tensor_tensor(
                xn[:], src_psum, rinv[:], scale_sb,
                mybir.AluOpType.mult, mybir.AluOpType.mult)
            # rope
            bi = simap[si]
            co = cos_all[:, bi * HALF:(bi + 1) * HALF]
            sn = sin_all[:, bi * HALF:(bi + 1) * HALF]
            x1 = xn[:, 0:HALF]
            x2 = xn[:, HALF:H]
            t1 = work.tile([P, HALF], F32, tag="t1")
            t2 = work.tile([P, HALF], F32, tag="t2")
            xr = work.tile([P, H], MM_DT, tag="xr" + tag)
            nc.vector.tensor_mul(t1[:], x1, co)
            nc.vector.tensor_mul(t2[:], x2, sn)
            nc.vector.tensor_sub(xr[:, 0:HALF], t1[:], t2[:])
            nc.vector.tensor_mul(t1[:], x2, co)
            nc.vector.tensor_mul(t2[:], x1, sn)
            nc.vector.tensor_add(xr[:, HALF:H], t1[:], t2[:])
            return xr

        def transpose_out(pairs, psum_t):
            """Batch PE transposes back-to-back (each is_transpose mode
            switch costs ~120ns); evacuate on ACT (DVE is A's bottleneck)."""
            pts = []
            for i, (xr, dst_sb) in enumerate(pairs):
                pt = psum_t.tile([P, P], MM_DT, tag=f"pt{i}")
                nc.tensor.transpose(pt[:], xr[:], ident)
                pts.append(pt)
            for (xr, dst_sb), pt in zip(pairs, pts):
                nc.scalar.copy(dst_sb, pt[:])

        # ---- Phase A: merged K/V/Q projection, norm+rope, transposes ----
        # Noise chunks (sc >= 8) also produce the 2 q-heads from the same
        # staged x tiles: moving = [Wk|Wv|Wq0|Wq1] (512 wide).
        xp = ctx.enter_context(tc.tile_pool(name=pfx + "pa_x", bufs=3))
        work = ctx.enter_context(tc.tile_pool(name=pfx + "pa_w", bufs=2))
        # Noise chunks first: their triple (k,q0,q1) norm chains are the DVE
        # bottleneck, so front-load them; the lighter ctx chains then gate
        # the A->C transition. Rope-table group 1 (ctx tiles 8..31) is built
        # after 2 ctx chunks so early chains aren't queued behind it on DVE.
        CHUNK_ORDER = list(range(NOISE_CHUNK0, S_CHUNKS)) + \
            list(range(0, NOISE_CHUNK0))
        with tc.tile_pool(name=pfx + "pa_ps", bufs=2, space="PSUM") as pskv, \
             tc.tile_pool(name=pfx + "pa_pt", bufs=1, space="PSUM") as pst:
            # PE warm-up spin while the first x-chunk DMA is in flight:
            # ~2us of sustained matmuls starts flipping the HAM clock-gate
            # to 8/8 before the real projection matmuls arrive.
            warm = pskv.tile([P, 4 * H], F32, tag="kv0", name="pswarm")
            for _ in range(18):
                nc.tensor.matmul(warm[:, 0:P], ident, ident,
                                 start=True, stop=True)
            for ci, sc in enumerate(CHUNK_ORDER):
                if ci == 6:
                    emit_rope_build(BUILD_G1, len(BUILD_G0))
                noise = sc >= NOISE_CHUNK0
                W = 4 * H if noise else 2 * H
                xt = xp.tile([P, D_TILES, FREE], MM_DT, tag="xstage",
                             name="xstage")
                nc.sync.dma_start(
                    xt[:], xTr[:, :, sc * FREE:(sc + 1) * FREE])
                # two half-chunks of 256 tokens, each double-buffered in PSUM
                for half in range(2):
                    ps = [pskv.tile([P, 4 * H], F32, tag=f"kv{j}",
                                    name=f"pskv{j}") for j in range(2)]
                    for d in range(D_TILES):
                        wslice = wkvq_sb[:, d * 4 * H:d * 4 * H + W]
                        for j in range(2):
                            tok0 = half * 2 * P + j * P
                            nc.tensor.matmul(
                                ps[j][:, 0:W],
                                xt[:, d, tok0:tok0 + P],
                                wslice,
                                start=(d == 0), stop=(d == D_TILES - 1))
                    for j in range(2):
                        si = sc * 4 + half * 2 + j
                        nc.scalar.copy(
                            v_sb[:, si * P:(si + 1) * P], ps[j][:, H:2 * H])
                        pairs = [(norm_rope(ps[j][:, 0:H], ksc_sb, si,
                                            work, "k"),
                                  kT_sb[:, si * P:(si + 1) * P])]
                        if noise:
                            ti = si - NOISE_TILE0
                            for hh in range(HEADS_PER_CORE):
                                pairs.append((
                                    norm_rope(ps[j][:, (2 + hh) * H:(3 + hh) * H],
                                              qsc_sb, si, work, f"q{hh}"),
                                    qT_sb[:, hh * T_NOISE + ti * P:
                                          hh * T_NOISE + (ti + 1) * P]))
                        transpose_out(pairs, pst)

        nc.sync.dma_start(wo_sb[:], wo[:])

        # ---- Phase C+D fused: attention, then per-t-chunk o-projection ----
        # Software-pipelined with LAG=2: AV/rowsum of score-pair p-2 are
        # emitted AFTER the score matmuls of pair p, so the in-order PE queue
        # never stalls waiting for exp on ACT (keeps HAM at K=8/8 too).
        # After both heads of a t-chunk finish, that chunk's o-projection
        # (phase D work) is drip-fed into the PE stream one t-tile at a time.
        # PSUM budget: scores 2x2 + av 1 + rr 1 + po 2 = 8 banks exactly.
        PAIR = 2 * FREE   # exp processes two score banks at once
        NSP = S_TILES // 2
        LAG = 2
        pexp = ctx.enter_context(tc.tile_pool(name=pfx + "pc_exp", bufs=4))
        dwork = ctx.enter_context(tc.tile_pool(name=pfx + "pd_w", bufs=3))
        with tc.tile_pool(name=pfx + "pc_av", bufs=1, space="PSUM") as pav, \
             tc.tile_pool(name=pfx + "pc_r", bufs=1, space="PSUM") as pr, \
             tc.tile_pool(name=pfx + "pd_ps", bufs=1, space="PSUM") as pso:

            def emit_avrr(st):
                ex, sp, av, rr = st["ex"], st["sp"], st["av"], st["rr"]
                for u in range(2):
                    si = sp * 2 + u
                    nc.tensor.matmul(
                        av[:], v_sb[:, si * P:(si + 1) * P],
                        ex[:, u * FREE:(u + 1) * FREE],
                        start=(si == 0), stop=(si == S_TILES - 1))
                for u in range(2):
                    si = sp * 2 + u
                    nc.tensor.matmul(
                        rr[:], ones,
                        ex[:, u * FREE:(u + 1) * FREE],
                        start=(si == 0), stop=(si == S_TILES - 1))

            def finish_chunk(st):
                hh, tch = st["hh"], st["tch"]
                nc.vector.tensor_copy(
                    oT_sb[:, hh * T_NOISE + tch * FREE:
                          hh * T_NOISE + (tch + 1) * FREE], st["av"][:])
                nc.vector.tensor_copy(
                    r_all[0:1, hh * T_NOISE + tch * FREE:
                          hh * T_NOISE + (tch + 1) * FREE], st["rr"][0:1, :])

            def prep_rcol(tch):
                # raw row-sums -> per-partition columns (tiny SBUF->SBUF DMA
                # transposes), then one cheap reciprocal per head slice
                # scalar (ACT) hwdge queue: keeps these off the sync queue
                # where the output DMAs live
                for hh in range(HEADS_PER_CORE):
                    for ti in range(tch * 4, tch * 4 + 4):
                        nc.scalar.dma_start(
                            rcol[:, hh * T_TILES + ti:hh * T_TILES + ti + 1],
                            r_all[0:1, hh * T_NOISE + ti * P:
                                  hh * T_NOISE + (ti + 1) * P])
                    c0 = hh * T_TILES + tch * 4
                    nc.vector.reciprocal(rcol[:, c0:c0 + 4], rcol[:, c0:c0 + 4])

            def emit_oproj_unit(ti, dc, pool=None, tail=False):
                # o-projection for one (128-token tile, 512-col d-chunk):
                # tmp = po1 * r1; ot = po0 * r0 + tmp; bf16 out.
                # Units are dripped >= 2 score-pairs apart so the pso bufs=1
                # WAR (next unit's matmul vs this unit's DVE reads) clears.
                # In the tail, tmp moves to ACT (idle there) and po comes from
                # the 4-bank ptail pool so units pipeline.
                pool = pool if pool is not None else pso
                po = [pool.tile([P, FREE], F32, tag=f"po{h}", name=f"po{h}")
                      for h in range(HEADS_PER_CORE)]
                for h in range(HEADS_PER_CORE):
                    nc.tensor.matmul(
                        po[h][:],
                        oT_sb[:, h * T_NOISE + ti * P:h * T_NOISE + (ti + 1) * P],
                        wo_sb[:, h * D + dc * FREE:h * D + (dc + 1) * FREE],
                        start=True, stop=True)
                tmp = dwork.tile([P, FREE], F32, tag="tmp")
                if tail:
                    nc.scalar.activation(
                        tmp[:], po[1][:], mybir.ActivationFunctionType.Copy,
                        scale=rcol[:, T_TILES + ti:T_TILES + ti + 1])
                else:
                    nc.vector.tensor_scalar_mul(
                        tmp[:], po[1][:],
                        rcol[:, T_TILES + ti:T_TILES + ti + 1])
                ot = dwork.tile([P, FREE], MM_DT, tag="ot")
                nc.vector.scalar_tensor_tensor(
                    ot[:], po[0][:], rcol[:, ti:ti + 1], tmp[:],
                    mybir.AluOpType.mult, mybir.AluOpType.add)
                nc.sync.dma_start(
                    out[ti * P:(ti + 1) * P, dc * FREE:(dc + 1) * FREE],
                    ot[:])

            pending = []          # score-pairs awaiting AV/rowsum emission
            dqueue = []           # o-projection (ti, dc) units awaiting emission

            def retire_one():
                st = pending.pop(0)
                emit_avrr(st)
                if st["sp"] == NSP - 1:
                    finish_chunk(st)
                    if st["hh"] == HEADS_PER_CORE - 1:
                        prep_rcol(st["tch"])
                        dqueue.extend(
                            (ti, dc)
                            for ti in range(st["tch"] * 4, st["tch"] * 4 + 4)
                            for dc in range(D_CHUNKS))

            gpair = 0
            with tc.tile_pool(name=pfx + "pc_sc", bufs=2,
                              space="PSUM") as psc:
                for tch in range(T_CHUNKS):
                    for hh in range(HEADS_PER_CORE):
                        qslice = qT_sb[:, hh * T_NOISE + tch * FREE:
                                       hh * T_NOISE + (tch + 1) * FREE]
                        av = pav.tile([P, FREE], F32, tag="av")
                        rr = pr.tile([P, FREE], F32, tag="rr")
                        for sp in range(NSP):
                            sc_ps = psc.tile([P, PAIR], F32, tag="sc")
                            ex = pexp.tile([P, PAIR], MM_DT, tag="ex")
                            for u in range(2):
                                si = sp * 2 + u
                                nc.tensor.matmul(
                                    sc_ps[:, u * FREE:(u + 1) * FREE],
                                    kT_sb[:, si * P:(si + 1) * P], qslice,
                                    start=True, stop=True)
                            if len(pending) >= LAG:
                                retire_one()
                            if dqueue and gpair % 2 == 0:
                                emit_oproj_unit(*dqueue.pop(0))
                            nc.scalar.activation(
                                ex[:], sc_ps[:],
                                mybir.ActivationFunctionType.Exp,
                                scale=INV_SQRT_H)
                            pending.append(dict(ex=ex, sp=sp, av=av, rr=rr,
                                                hh=hh, tch=tch))
                            gpair += 1
                while pending:
                    retire_one()
            # tail o-projection units reuse the freed score banks (4-bank
            # double-buffered pool) so consecutive units pipeline
            with tc.tile_pool(name=pfx + "pd_tail", bufs=2,
                              space="PSUM") as ptail:
                for ti, dc in dqueue:
                    emit_oproj_unit(ti, dc, pool=ptail, tail=True)


def _get_program(reps=1):
    key = f"prog{reps}"
    if key not in _CACHE:
        _CACHE[key] = _build_program(reps)
    return _CACHE[key]


def prepare_in_maps(x_noise, target_hidden, Wq, Wk, Wv, Wo, q_scale, k_scale,
                    noise_positions, ctx_positions):
    import ml_dtypes
    bf16 = ml_dtypes.bfloat16

    x_noise = np.asarray(x_noise, dtype=np.float32)
    target_hidden = np.asarray(target_hidden, dtype=np.float32)
    Wq = np.asarray(Wq, dtype=np.float32)
    Wk = np.asarray(Wk, dtype=np.float32)
    Wv = np.asarray(Wv, dtype=np.float32)
    Wo = np.asarray(Wo, dtype=np.float32)
    q_scale = np.asarray(q_scale, dtype=np.float32)
    k_scale = np.asarray(k_scale, dtype=np.float32)

    x_all = np.concatenate([target_hidden, x_noise], axis=0)       # (S, D)
    # xTr[p, dt, s] = x_all[s, dt*128+p]
    xTr = np.ascontiguousarray(
        x_all.T.reshape(D_TILES, P, S_ALL).transpose(1, 0, 2)).astype(bf16)

    pos_all = np.concatenate(
        [np.asarray(ctx_positions), np.asarray(noise_positions)]
    ).astype(np.float32)
    # pos columns: consts[:, CO_POS + si] = pos of tile si (same for all p? no:
    # partition p holds position of token si*128+p)
    posT = np.ascontiguousarray(pos_all.reshape(S_TILES, P).T)     # (P, 48)
    inv_freq = (ROPE_THETA ** (-np.arange(HALF, dtype=np.float32) * 2.0 / H)
                ).astype(np.float32)
    constsf = np.zeros((P, CONST_W), dtype=np.float32)
    constsf[:, CO_INVF:CO_INVF + HALF] = inv_freq
    constsf[:, CO_QSC:CO_QSC + H] = q_scale
    constsf[:, CO_KSC:CO_KSC + H] = k_scale
    constsf[:, CO_POS:CO_POS + S_TILES] = posT

    constsb = np.zeros((P, 2 * P), dtype=np.float32)
    constsb[:, 0:P] = np.eye(P, dtype=np.float32)
    constsb[:, P:2 * P] = 1.0
    constsb = constsb.astype(bf16)

    in_maps = []
    for c in range(N_CORES):
        # wkvq[p, dt*512 + col] = W*[dt*128+p, col] with cols [k|v|q0|q1]
        wkvq = np.concatenate([
            Wk[:, c, :], Wv[:, c, :],
            Wq[:, 2 * c, :], Wq[:, 2 * c + 1, :]], axis=1)          # (D, 512)
        wkvq = np.ascontiguousarray(
            wkvq.reshape(D_TILES, P, 4 * H).transpose(1, 0, 2)
            .reshape(P, D_TILES * 4 * H)).astype(bf16)
        # wo[p, h*D + dcol] = Wo[2c+h, p, dcol]
        woc = np.ascontiguousarray(
            Wo[2 * c:2 * c + 2].transpose(1, 0, 2).reshape(P, 2 * D)
        ).astype(bf16)
        in_maps.append({
            "xTr": xTr, "wkvq": wkvq, "wo": woc,
            "constsf": constsf, "constsb": constsb,
        })
    return in_maps


def kernel(**inputs):
    in_maps = prepare_in_maps(**inputs)
    nc, out_name = _get_program()
    res = run_bass_kernel_spmd(nc, in_maps, core_ids=list(range(N_CORES)))
    acc = np.zeros((T_NOISE, D), dtype=np.float32)
    for r in res.results:
        acc += np.asarray(r[out_name], dtype=np.float32)
    return acc


def run_traced(inputs, **kw):
    """Run once with NTFF tracing; returns BassKernelResults (exec_time_ns)."""
    in_maps = prepare_in_maps(**inputs)
    nc, out_name = _get_program()
    return run_bass_kernel_spmd(nc, in_maps, core_ids=list(range(N_CORES)),
                                trace=True, **kw)


# revision 38
# speedup vs baseline: 1.0314x; 1.0028x over previous
"""DFlashAttention Trainium2 kernel (8-core tensor-parallel over attention heads).

Shapes (hardcoded): D=2048, N=16 q-heads, K=8 kv-heads, H=128,
T_NOISE=2048 (query tokens), T_CTX=4096, S=6144 (kv tokens).

Sharding: core c owns q-heads {2c, 2c+1} and kv-head c (GQA groups=2).
Each core computes a partial (T, D) output (its 2 heads' slice of the
o-projection contraction); the host sums the 8 partials (TP unshard).

v2 layout strategy per core (all matmul operands bf16, fp32 PSUM):
  - x^T packed host-side as [128, 16 d-tiles, S]; ONE 2MB DMA per 512-token
    chunk (split across all 16 SDMA engines).
  - merged projection: per chunk, stationary = x-tile [128d, 128tok],
    moving = [Wk|Wv] (ctx chunks, 256) or [Wk|Wv|Wq01] (noise chunks, 512).
    Q projection rides the same x tiles => no separate phase B.
  - RMSNorm over H via ACT Square+accum_out; RoPE via on-device sin/cos
    (angle mod 2pi + range wrap + ACT Sin); tables built once for all 48
    token tiles; build overlaps phase A (pools never reuse its space).
  - per 128-token tile: norm+rope (fp32) -> bf16 -> PE transpose -> kT/qT.
  - attention in [s, t] orientation: scores^T = kT.T @ qT; exp on ACT
    (scale=1/sqrt(H) folded; no max subtraction, |score| <= 13.8, fp32-safe);
    probs bf16; row-sums via ones-matmul accumulated in PSUM; A@V accumulates
    over s-tiles in PSUM with V in natural [s, h] layout.
  - softmax division deferred past the o-projection (denominator constant
    along the contraction), applied as per-partition scalar multiply.
"""

import sys

for _p in ("/opt/trn_rl_repo", "/root/.axon_site/_ro/trn_rl_repo"):
    if _p not in sys.path:
        sys.path.append(_p)

import math
import numpy as np

import concourse.bass as bass
import concourse.tile as tile
from concourse import bacc
from concourse import mybir
from concourse.bass_utils import run_bass_kernel_spmd

D = 2048
N_HEADS = 16
K_HEADS = 8
H = 128
T_NOISE = 2048
T_CTX = 4096
S_ALL = T_CTX + T_NOISE          # 6144
EPS = 1e-6
ROPE_THETA = 1e6
N_CORES = 8
HEADS_PER_CORE = N_HEADS // N_CORES   # 2

P = 128                       # partition dim
S_TILES = S_ALL // P          # 48
T_TILES = T_NOISE // P        # 16
NOISE_TILE0 = T_CTX // P      # 32  (noise tokens are s-tiles 32..47)
D_TILES = D // P              # 16
FREE = 512                    # moving free-dim chunk
T_CHUNKS = T_NOISE // FREE    # 4
S_CHUNKS = S_ALL // FREE      # 12
NOISE_CHUNK0 = T_CTX // FREE  # 8
D_CHUNKS = D // FREE          # 4

F32 = mybir.dt.float32
BF16 = mybir.dt.bfloat16
MM_DT = BF16                  # dtype for all matmul operands

TWO_PI = 2.0 * math.pi
INV_SQRT_H = 1.0 / math.sqrt(H)
HALF = H // 2

# consts tensor layout (fp32, [P, CONST_W])
CO_INVF = 0                  # [P, 64]   inv_freq broadcast
CO_QSC = 64                  # [P, 128]  q_scale broadcast
CO_KSC = 192                 # [P, 128]  k_scale broadcast
CO_POS = 320                 # [P, 48]   positions, tile-major columns
CONST_W = 368

_CACHE = {}


def _build_program(reps=1):
    nc = bacc.Bacc("TRN2", target_bir_lowering=False, debug=False,
                   num_devices=N_CORES)

    xTr = nc.dram_tensor("xTr", [P, D_TILES, S_ALL], MM_DT,
                         kind="ExternalInput").ap()
    wkvq = nc.dram_tensor("wkvq", [P, D_TILES * 4 * H], MM_DT,
                          kind="ExternalInput").ap()
    wo = nc.dram_tensor("wo", [P, HEADS_PER_CORE * D], MM_DT,
                        kind="ExternalInput").ap()
    constsf = nc.dram_tensor("constsf", [P, CONST_W], F32,
                             kind="ExternalInput").ap()
    constsb = nc.dram_tensor("constsb", [P, 2 * P], MM_DT,
                             kind="ExternalInput").ap()
    out = nc.dram_tensor("out", [T_NOISE, D], MM_DT, kind="ExternalOutput").ap()

    with tile.TileContext(nc) as tc:
        for rep in range(reps):
            _emit(nc, tc, xTr, wkvq, wo, constsf, constsb, out,
                  pfx=f"r{rep}_")
    nc.compile()
    return nc, "out"


def _emit(nc, tc, xTr, wkvq, wo, constsf, constsb, out, pfx=""):
    import contextlib
    ctx = contextlib.ExitStack()
    with ctx:
        const = ctx.enter_context(tc.tile_pool(name=pfx + "const", bufs=1))
        persist = ctx.enter_context(tc.tile_pool(name=pfx + "persist", bufs=1))

        # ---- constants (3 DMAs total) ----
        cf = const.tile([P, CONST_W], F32, tag="cf")
        nc.sync.dma_start(cf[:], constsf[:])
        cb = const.tile([P, 2 * P], MM_DT, tag="cb")
        nc.sync.dma_start(cb[:], constsb[:])
        ident = cb[:, 0:P]
        # full [128,128] all-ones stationary for row-sums: M=128 output (all
        # rows equal) avoids the ~100ns PE reconfig cost of M=1 matmuls
        ones = cb[:, P:2 * P]
        invf_sb = cf[:, CO_INVF:CO_INVF + HALF]
        qsc_sb = cf[:, CO_QSC:CO_QSC + H]
        ksc_sb = cf[:, CO_KSC:CO_KSC + H]
        pos_sb = cf[:, CO_POS:CO_POS + S_TILES]
        eps_col = const.tile([P, 1], F32, tag="eps")
        nc.vector.memset(eps_col[:], EPS)

        wkvq_sb = const.tile([P, D_TILES * 4 * H], MM_DT, tag="wkvq")
        nc.sync.dma_start(wkvq_sb[:], wkvq[:])
        # wo is only needed in the o-projection; its DMA is emitted after
        # phase A so it doesn't delay the first x-chunk on the sync queue.
        wo_sb = const.tile([P, HEADS_PER_CORE * D], MM_DT, tag="wo")

        # ---- persistent activations ----
        sin_all = persist.tile([P, S_TILES * HALF], F32, tag="sin")
        cos_all = persist.tile([P, S_TILES * HALF], F32, tag="cos")
        kT_sb = persist.tile([P, S_ALL], MM_DT, tag="kT")
        v_sb = persist.tile([P, S_ALL], MM_DT, tag="v")       # [s-tile, h] blocks
        qT_sb = persist.tile([P, HEADS_PER_CORE * T_NOISE], MM_DT, tag="qT")
        oT_sb = persist.tile([P, HEADS_PER_CORE * T_NOISE], MM_DT, tag="oT")
        r_all = persist.tile([1, HEADS_PER_CORE * T_NOISE], F32, tag="r")
        rcol = persist.tile([P, HEADS_PER_CORE * T_TILES], F32, tag="rcol")

        # ---- RoPE sin/cos tables for all 48 token tiles ----
        # angle = pos * inv_freq; range-reduce mod 2pi via Cody-Waite
        # (k = int(angle/2pi); red = ((ang - k*c1) - k*c2) - k*c3).
        # Pool stays open for the whole kernel so phase A never waits on a
        # space-reuse (WAR) dependency against the build. Tables are stored
        # in BUILD order (noise tiles first, matching the noise-first chunk
        # order); `simap` maps token-tile index -> table column block.
        CW1, CW2, CW3 = 6.28125, 0.0019353071693331003, 1.0253131677018246e-11
        BUILD_G0 = list(range(NOISE_TILE0, S_TILES)) + list(range(0, 8))
        BUILD_G1 = list(range(8, NOISE_TILE0))
        simap = {}
        for idx, si in enumerate(BUILD_G0 + BUILD_G1):
            simap[si] = idx
        rp = ctx.enter_context(tc.tile_pool(name=pfx + "ropebuild", bufs=1))

        def emit_rope_build(tiles, dst0):
            ng = len(tiles)
            ang = rp.tile([P, ng * HALF], F32, tag="ang", name="ang")
            kq = rp.tile([P, ng * HALF], F32, tag="kq", name="kq")
            ki = rp.tile([P, ng * HALF], mybir.dt.int32, tag="ki", name="ki")
            wrap = rp.tile([P, ng * HALF], F32, tag="wrap", name="wrap")
            for j, si in enumerate(tiles):
                nc.vector.tensor_scalar_mul(
                    ang[:, j * HALF:(j + 1) * HALF], invf_sb,
                    pos_sb[:, si:si + 1])
            nc.vector.tensor_scalar_mul(kq[:], ang[:], 1.0 / TWO_PI)
            nc.vector.tensor_copy(ki[:], kq[:])
            nc.vector.tensor_copy(kq[:], ki[:])
            nc.vector.cody_waite_cascade(ang[:], ang[:], kq[:], CW1, CW2, CW3)
            dst = slice(dst0 * HALF, (dst0 + ng) * HALF)
            nc.vector.add_range_wrap(wrap[:], ang[:], 0.0, math.pi, TWO_PI)
            nc.scalar.activation(sin_all[:, dst], wrap[:],
                                 mybir.ActivationFunctionType.Sin)
            nc.vector.add_range_wrap(wrap[:], ang[:], math.pi / 2, math.pi,
                                     TWO_PI)
            nc.scalar.activation(cos_all[:, dst], wrap[:],
                                 mybir.ActivationFunctionType.Sin)

        emit_rope_build(BUILD_G0, 0)

        def norm_rope(src_psum, scale_sb, si, work, tag):
            """src_psum [P(tok),H] fp32 -> rms-norm*scale -> rope -> bf16
            xr tile. si = token-tile index for positions."""
            sq = work.tile([P, H], F32, tag="sq")
            ssq = work.tile([P, 1], F32, tag="ssq")
            nc.scalar.activation(sq[:], src_psum,
                                 mybir.ActivationFunctionType.Square,
                                 accum_out=ssq[:])
            rms = work.tile([P, 1], F32, tag="rms")
            nc.scalar.activation(rms[:], ssq[:],
                                 mybir.ActivationFunctionType.Sqrt,
                                 bias=eps_col[:], scale=1.0 / H)
            rinv = work.tile([P, 1], F32, tag="rinv")
            nc.vector.reciprocal(rinv[:], rms[:])
            xn = work.tile([P, H], F32, tag="xn")
            nc.vector.scalar_tensor_tensor(
                xn[:], src_psum, rinv[:], scale_sb,
                mybir.AluOpType.mult, mybir.AluOpType.mult)
            # rope
            bi = simap[si]
            co = cos_all[:, bi * HALF:(bi + 1) * HALF]
            sn = sin_all[:, bi * HALF:(bi + 1) * HALF]
            x1 = xn[:, 0:HALF]
            x2 = xn[:, HALF:H]
            t1 = work.tile([P, HALF], F32, tag="t1")
            t2 = work.tile([P, HALF], F32, tag="t2")
            xr = work.tile([P, H], MM_DT, tag="xr" + tag)
            nc.vector.tensor_mul(t1[:], x1, co)
            nc.vector.tensor_mul(t2[:], x2, sn)
            nc.vector.tensor_sub(xr[:, 0:HALF], t1[:], t2[:])
            nc.vector.tensor_mul(t1[:], x2, co)
            nc.vector.tensor_mul(t2[:], x1, sn)
            nc.vector.tensor_add(xr[:, HALF:H], t1[:], t2[:])
            return xr

        def transpose_out(pairs, psum_t):
            """Batch PE transposes back-to-back (each is_transpose mode
            switch costs ~120ns); evacuate on ACT (DVE is A's bottleneck)."""
            pts = []
            for i, (xr, dst_sb) in enumerate(pairs):
                pt = psum_t.tile([P, P], MM_DT, tag=f"pt{i}")
                nc.tensor.transpose(pt[:], xr[:], ident)
                pts.append(pt)
            for (xr, dst_sb), pt in zip(pairs, pts):
                nc.scalar.copy(dst_sb, pt[:])

        # ---- Phase A: merged K/V/Q projection, norm+rope, transposes ----
        # Noise chunks (sc >= 8) also produce the 2 q-heads from the same
        # staged x tiles: moving = [Wk|Wv|Wq0|Wq1] (512 wide).
        xp = ctx.enter_context(tc.tile_pool(name=pfx + "pa_x", bufs=3))
        work = ctx.enter_context(tc.tile_pool(name=pfx + "pa_w", bufs=2))
        # Noise chunks first: their triple (k,q0,q1) norm chains are the DVE
        # bottleneck, so front-load them; the lighter ctx chains then gate
        # the A->C transition. Rope-table group 1 (ctx tiles 8..31) is built
        # after 2 ctx chunks so early chains aren't queued behind it on DVE.
        CHUNK_ORDER = list(range(NOISE_CHUNK0, S_CHUNKS)) + \
            list(range(0, NOISE_CHUNK0))
        with tc.tile_pool(name=pfx + "pa_ps", bufs=2, space="PSUM") as pskv, \
             tc.tile_pool(name=pfx + "pa_pt", bufs=1, space="PSUM") as pst:
            # PE warm-up spin while the first x-chunk DMA is in flight:
            # ~2us of sustained matmuls starts flipping the HAM clock-gate
            # to 8/8 before the real projection matmuls arrive.
            warm = pskv.tile([P, 4 * H], F32, tag="kv0", name="pswarm")
            for _ in range(18):
                nc.tensor.matmul(warm[:, 0:P], ident, ident,
                                 start=True, stop=True)
            for ci, sc in enumerate(CHUNK_ORDER):
                if ci == 6:
                    emit_rope_build(BUILD_G1, len(BUILD_G0))
                noise = sc >= NOISE_CHUNK0
                W = 4 * H if noise else 2 * H
                xt = xp.tile([P, D_TILES, FREE], MM_DT, tag="xstage",
                             name="xstage")
                nc.sync.dma_start(
                    xt[:], xTr[:, :, sc * FREE:(sc + 1) * FREE])
                # two half-chunks of 256 tokens, each double-buffered in PSUM
                for half in range(2):
                    ps = [pskv.tile([P, 4 * H], F32, tag=f"kv{j}",
                                    name=f"pskv{j}") for j in range(2)]
                    for d in range(D_TILES):
                        wslice = wkvq_sb[:, d * 4 * H:d * 4 * H + W]
                        for j in range(2):
                            tok0 = half * 2 * P + j * P
                            nc.tensor.matmul(
                                ps[j][:, 0:W],
                                xt[:, d, tok0:tok0 + P],
                                wslice,
                                start=(d == 0), stop=(d == D_TILES - 1))
                    for j in range(2):
                        si = sc * 4 + half * 2 + j
                        nc.scalar.copy(
                            v_sb[:, si * P:(si + 1) * P], ps[j][:, H:2 * H])
                        pairs = [(norm_rope(ps[j][:, 0:H], ksc_sb, si,
                                            work, "k"),
                                  kT_sb[:, si * P:(si + 1) * P])]
                        if noise:
                            ti = si - NOISE_TILE0
                            for hh in range(HEADS_PER_CORE):
                                pairs.append((
                                    norm_rope(ps[j][:, (2 + hh) * H:(3 + hh) * H],
                                              qsc_sb, si, work, f"q{hh}"),
                                    qT_sb[:, hh * T_NOISE + ti * P:
                                          hh * T_NOISE + (ti + 1) * P]))
                        transpose_out(pairs, pst)

        nc.sync.dma_start(wo_sb[:], wo[:])

        # ---- Phase C+D fused: attention, then per-t-chunk o-projection ----
        # Software-pipelined with LAG=2: AV/rowsum of score-pair p-2 are
        # emitted AFTER the score matmuls of pair p, so the in-order PE queue
        # never stalls waiting for exp on ACT (keeps HAM at K=8/8 too).
        # After both heads of a t-chunk finish, that chunk's o-projection
        # (phase D work) is drip-fed into the PE stream one t-tile at a time.
        # PSUM budget: scores 2x2 + av 1 + rr 1 + po 2 = 8 banks exactly.
        PAIR = 2 * FREE   # exp processes two score banks at once
        NSP = S_TILES // 2
        LAG = 3
        pexp = ctx.enter_context(tc.tile_pool(name=pfx + "pc_exp", bufs=4))
        dwork = ctx.enter_context(tc.tile_pool(name=pfx + "pd_w", bufs=3))
        with tc.tile_pool(name=pfx + "pc_av", bufs=1, space="PSUM") as pav, \
             tc.tile_pool(name=pfx + "pc_r", bufs=1, space="PSUM") as pr, \
             tc.tile_pool(name=pfx + "pd_ps", bufs=1, space="PSUM") as pso:

            def emit_avrr(st):
                ex, sp, av, rr = st["ex"], st["sp"], st["av"], st["rr"]
                for u in range(2):
                    si = sp * 2 + u
                    nc.tensor.matmul(
                        av[:], v_sb[:, si * P:(si + 1) * P],
                        ex[:, u * FREE:(u + 1) * FREE],
                        start=(si == 0), stop=(si == S_TILES - 1))
                for u in range(2):
                    si = sp * 2 + u
                    nc.tensor.matmul(
                        rr[:], ones,
                        ex[:, u * FREE:(u + 1) * FREE],
                        start=(si == 0), stop=(si == S_TILES - 1))

            def finish_chunk(st):
                hh, tch = st["hh"], st["tch"]
                nc.vector.tensor_copy(
                    oT_sb[:, hh * T_NOISE + tch * FREE:
                          hh * T_NOISE + (tch + 1) * FREE], st["av"][:])
                nc.vector.tensor_copy(
                    r_all[0:1, hh * T_NOISE + tch * FREE:
                          hh * T_NOISE + (tch + 1) * FREE], st["rr"][0:1, :])

            def prep_rcol(tch):
                # raw row-sums -> per-partition columns (tiny SBUF->SBUF DMA
                # transposes), then one cheap reciprocal per head slice
                # scalar (ACT) hwdge queue: keeps these off the sync queue
                # where the output DMAs live
                for hh in range(HEADS_PER_CORE):
                    for ti in range(tch * 4, tch * 4 + 4):
                        nc.scalar.dma_start(
                            rcol[:, hh * T_TILES + ti:hh * T_TILES + ti + 1],
                            r_all[0:1, hh * T_NOISE + ti * P:
                                  hh * T_NOISE + (ti + 1) * P])
                    c0 = hh * T_TILES + tch * 4
                    nc.vector.reciprocal(rcol[:, c0:c0 + 4], rcol[:, c0:c0 + 4])

            def emit_oproj_unit(ti, dc, pool=None, tail=False):
                # o-projection for one (128-token tile, 512-col d-chunk):
                # tmp = po1 * r1; ot = po0 * r0 + tmp; bf16 out.
                # Units are dripped >= 2 score-pairs apart so the pso bufs=1
                # WAR (next unit's matmul vs this unit's DVE reads) clears.
                # In the tail, tmp moves to ACT (idle there) and po comes from
                # the 4-bank ptail pool so units pipeline.
                pool = pool if pool is not None else pso
                po = [pool.tile([P, FREE], F32, tag=f"po{h}", name=f"po{h}")
                      for h in range(HEADS_PER_CORE)]
                for h in range(HEADS_PER_CORE):
                    nc.tensor.matmul(
                        po[h][:],
                        oT_sb[:, h * T_NOISE + ti * P:h * T_NOISE + (ti + 1) * P],
                        wo_sb[:, h * D + dc * FREE:h * D + (dc + 1) * FREE],
                        start=True, stop=True)
                tmp = dwork.tile([P, FREE], F32, tag="tmp")
                if tail:
                    nc.scalar.activation(
                        tmp[:], po[1][:], mybir.ActivationFunctionType.Copy,
                        scale=rcol[:, T_TILES + ti:T_TILES + ti + 1])
                else:
                    nc.vector.tensor_scalar_mul(
                        tmp[:], po[1][:],
                        rcol[:, T_TILES + ti:T_TILES + ti + 1])
                ot = dwork.tile([P, FREE], MM_DT, tag="ot")
                nc.vector.scalar_tensor_tensor(
                    ot[:], po[0][:], rcol[:, ti:ti + 1], tmp[:],
                    mybir.AluOpType.mult, mybir.AluOpType.add)
                nc.sync.dma_start(
                    out[ti * P:(ti + 1) * P, dc * FREE:(dc + 1) * FREE],
                    ot[:])

            pending = []          # score-pairs awaiting AV/rowsum emission
            dqueue = []           # o-projection (ti, dc) units awaiting emission

            def retire_one():
                st = pending.pop(0)
                emit_avrr(st)
                if st["sp"] == NSP - 1:
                    finish_chunk(st)
                    if st["hh"] == HEADS_PER_CORE - 1:
                        prep_rcol(st["tch"])
                        dqueue.extend(
                            (ti, dc)
                            for ti in range(st["tch"] * 4, st["tch"] * 4 + 4)
                            for dc in range(D_CHUNKS))

            gpair = 0
            with tc.tile_pool(name=pfx + "pc_sc", bufs=2,
                              space="PSUM") as psc:
                for tch in range(T_CHUNKS):
                    for hh in range(HEADS_PER_CORE):
                        qslice = qT_sb[:, hh * T_NOISE + tch * FREE:
                                       hh * T_NOISE + (tch + 1) * FREE]
                        av = pav.tile([P, FREE], F32, tag="av")
                        rr = pr.tile([P, FREE], F32, tag="rr")
                        for sp in range(NSP):
                            sc_ps = psc.tile([P, PAIR], F32, tag="sc")
                            ex = pexp.tile([P, PAIR], MM_DT, tag="ex")
                            for u in range(2):
                                si = sp * 2 + u
                                nc.tensor.matmul(
                                    sc_ps[:, u * FREE:(u + 1) * FREE],
                                    kT_sb[:, si * P:(si + 1) * P], qslice,
                                    start=True, stop=True)
                            if len(pending) >= LAG:
                                retire_one()
                            if dqueue and gpair % 2 == 0:
                                emit_oproj_unit(*dqueue.pop(0))
                            nc.scalar.activation(
                                ex[:], sc_ps[:],
                                mybir.ActivationFunctionType.Exp,
                                scale=INV_SQRT_H)
                            pending.append(dict(ex=ex, sp=sp, av=av, rr=rr,
                                                hh=hh, tch=tch))
                            gpair += 1
                while pending:
                    retire_one()
            # tail o-projection units reuse the freed score banks (4-bank
            # double-buffered pool) so consecutive units pipeline
            with tc.tile_pool(name=pfx + "pd_tail", bufs=2,
                              space="PSUM") as ptail:
                for ti, dc in dqueue:
                    emit_oproj_unit(ti, dc, pool=ptail, tail=True)


def _get_program(reps=1):
    key = f"prog{reps}"
    if key not in _CACHE:
        _CACHE[key] = _build_program(reps)
    return _CACHE[key]


def prepare_in_maps(x_noise, target_hidden, Wq, Wk, Wv, Wo, q_scale, k_scale,
                    noise_positions, ctx_positions):
    import ml_dtypes
    bf16 = ml_dtypes.bfloat16

    x_noise = np.asarray(x_noise, dtype=np.float32)
    target_hidden = np.asarray(target_hidden, dtype=np.float32)
    Wq = np.asarray(Wq, dtype=np.float32)
    Wk = np.asarray(Wk, dtype=np.float32)
    Wv = np.asarray(Wv, dtype=np.float32)
    Wo = np.asarray(Wo, dtype=np.float32)
    q_scale = np.asarray(q_scale, dtype=np.float32)
    k_scale = np.asarray(k_scale, dtype=np.float32)

    x_all = np.concatenate([target_hidden, x_noise], axis=0)       # (S, D)
    # xTr[p, dt, s] = x_all[s, dt*128+p]
    xTr = np.ascontiguousarray(
        x_all.T.reshape(D_TILES, P, S_ALL).transpose(1, 0, 2)).astype(bf16)

    pos_all = np.concatenate(
        [np.asarray(ctx_positions), np.asarray(noise_positions)]
    ).astype(np.float32)
    # pos columns: consts[:, CO_POS + si] = pos of tile si (same for all p? no:
    # partition p holds position of token si*128+p)
    posT = np.ascontiguousarray(pos_all.reshape(S_TILES, P).T)     # (P, 48)
    inv_freq = (ROPE_THETA ** (-np.arange(HALF, dtype=np.float32) * 2.0 / H)
                ).astype(np.float32)
    constsf = np.zeros((P, CONST_W), dtype=np.float32)
    constsf[:, CO_INVF:CO_INVF + HALF] = inv_freq
    constsf[:, CO_QSC:CO_QSC + H] = q_scale
    constsf[:, CO_KSC:CO_KSC + H] = k_scale
    constsf[:, CO_POS:CO_POS + S_TILES] = posT

    constsb = np.zeros((P, 2 * P), dtype=np.float32)
    constsb[:, 0:P] = np.eye(P, dtype=np.float32)
    constsb[:, P:2 * P] = 1.0
    constsb = constsb.astype(bf16)

    in_maps = []
    for c in range(N_CORES):
        # wkvq[p, dt*512 + col] = W*[dt*128+p, col] with cols [k|v|q0|q1]
        wkvq = np.concatenate([
            Wk[:, c, :], Wv[:, c, :],
            Wq[:, 2 * c, :], Wq[:, 2 * c + 1, :]], axis=1)          # (D, 512)
        wkvq = np.ascontiguousarray(
            wkvq.reshape(D_TILES, P, 4 * H).transpose(1, 0, 2)
            .reshape(P, D_TILES * 4 * H)).astype(bf16)
        # wo[p, h*D + dcol] = Wo[2c+h, p, dcol]
        woc = np.ascontiguousarray(
            Wo[2 * c:2 * c + 2].transpose(1, 0, 2).reshape(P, 2 * D)
        ).astype(bf16)
        in_maps.append({
            "xTr": xTr, "wkvq": wkvq, "wo": woc,
            "constsf": constsf, "constsb": constsb,
        })
    return in_maps


def kernel(**inputs):
    in_maps = prepare_in_maps(**inputs)
    nc, out_name = _get_program()
    res = run_bass_kernel_spmd(nc, in_maps, core_ids=list(range(N_CORES)))
    acc = np.zeros((T_NOISE, D), dtype=np.float32)
    for r in res.results:
        acc += np.asarray(r[out_name], dtype=np.float32)
    return acc


def run_traced(inputs, **kw):
    """Run once with NTFF tracing; returns BassKernelResults (exec_time_ns)."""
    in_maps = prepare_in_maps(**inputs)
    nc, out_name = _get_program()
    return run_bass_kernel_spmd(nc, in_maps, core_ids=list(range(N_CORES)),
                                trace=True, **kw)
